# revision 9
# baseline (speedup 1.0000x reference)
"""Trainium2 Bass kernel for nn_CATDecoderAttention (GQA causal attention block).

Computation (fp32 reference): q/k/v projections -> per-head RMSNorm on q,k ->
RoPE on q,k -> causal GQA attention (16 q heads, 4 kv heads, D=64) -> o-proj.

Sharding: sequence-parallel over queries. Each of the 8 cores owns 4 query
chunks of 128 rows (chunk set {8m+c : m=0..3} for core c), chosen so the causal
work is identical across cores (SPMD single program). A per-core permutation of
the 4096 t-rows (swapping blocks 8m+c <-> 8m+7 within each group of 8) puts the
core's own (diagonal) chunk at the fixed position 8m+7, so the diagonal
tri-mask sits at a compile-time program position; remaining causal differences
between cores are handled by a per-core bias table feeding exp() (whole-block
masking, free via the activation bias operand). K/V work is replicated across
cores; outputs are disjoint row-slices, so no collectives are needed, and the
host scatters rows back.

Layouts: projections consume host-pretransposed hidden^T and produce q/k in
[t, d] (norm/RoPE domain), then PE-transpose to [d, t] for QK^T. Scores are
computed transposed (S^T[j, i]) so PV needs no transpose; a constant ones
column appended to V yields the softmax denominators for free. fp32r matmuls
(1 cycle/row, ~1e-4 matmul error) everywhere except the PV stage (bf16).
"""

import sys
import numpy as np

sys.path.insert(0, "/opt/trn_rl_repo")

import ml_dtypes  # noqa: E402
from contextlib import ExitStack  # noqa: E402

import concourse.bass as bass  # noqa: E402
import concourse.tile as tile  # noqa: E402
from concourse import bacc, mybir  # noqa: E402
from concourse.bass_utils import run_bass_kernel_spmd  # noqa: E402

F32 = mybir.dt.float32
F32R = mybir.dt.float32r
BF16 = mybir.dt.bfloat16
AX = mybir.AxisListType.X
AF = mybir.ActivationFunctionType
ALU = mybir.AluOpType

B, T, HID = 1, 4096, 1024
H, HKV, D = 16, 4, 64
G = H // HKV
EPS = 1e-6
SCALE = 1.0 / np.sqrt(D)
NCORES = 8
NSLOT = 4           # own query chunks per core
NCH = T // 128      # 32 t-blocks
NEG = -30000.0      # exp(x + NEG) == 0 for any reachable score

# q head order in the q[t,d] working layout: pair p = heads (HORD[2p], HORD[2p+1])
# chosen so pair sides have kv groups (0,1) resp. (2,3) for QK row-tiling.
HORD = [0, 4, 1, 5, 2, 6, 3, 7, 8, 12, 9, 13, 10, 14, 11, 15]

PHASE_MARKS = []  # (phase_name, first_instruction_id) — filled by build_program


def _mark(nc, name):
    PHASE_MARKS.append((name, nc._state.next_id()))


def _bcast_free(tile_ap, idx, count, width):
    """AP reading tile_ap[:, idx, 0:width] broadcast `count` times along a
    middle free dim of step 0."""
    base = tile_ap[:, idx, 0:width]
    return bass.AP(tensor=base.tensor, offset=base.offset,
                   ap=[base.ap[0], [0, count], [1, width]])


def _norm_rope(nc, pool, x_td, tab, nh, eps_sb, out_dt=F32):
    """RMSNorm (eps inside sqrt, weight folded into tab) + RoPE.
    x_td: [128, nh, 64] f32 SBUF. tab: [128, 4, 32] f32 (C1|S1|S2|C2).
    Returns [128, nh, 64] rope'd tile."""
    sq = pool.tile([128, nh, 64], F32, tag="nr_sq")
    nc.vector.tensor_mul(sq, x_td, x_td)
    ss = pool.tile([128, nh], F32, tag="nr_ss")
    nc.vector.reduce_sum(ss, sq, axis=AX)
    st = pool.tile([128, nh], F32, tag="nr_st")
    nc.scalar.activation(st, ss, AF.Sqrt, bias=eps_sb, scale=1.0 / D)
    rs = pool.tile([128, nh], F32, tag="nr_rs")
    nc.vector.reciprocal(rs, st)
    nm = pool.tile([128, nh, 64], F32, tag="nr_nm")
    rs_b = bass.AP(tensor=rs.tensor, offset=rs.offset,
                   ap=[rs.ap[0], [1, nh], [0, 64]])
    nc.vector.tensor_mul(nm, x_td, rs_b)
    ro = pool.tile([128, nh, 64], out_dt, tag="nr_ro")
    ta = pool.tile([128, nh, 32], F32, tag="nr_ta")
    tb = pool.tile([128, nh, 32], F32, tag="nr_tb")
    x1 = nm[:, :, 0:32]
    x2 = nm[:, :, 32:64]
    c1 = _bcast_free(tab, 0, nh, 32)
    s1 = _bcast_free(tab, 1, nh, 32)
    s2 = _bcast_free(tab, 2, nh, 32)
    c2 = _bcast_free(tab, 3, nh, 32)
    nc.vector.tensor_mul(ta, x1, c1)
    nc.vector.tensor_mul(tb, x2, s1)
    nc.vector.tensor_sub(ro[:, :, 0:32], ta, tb)
    nc.vector.tensor_mul(ta, x1, s2)
    nc.vector.tensor_mul(tb, x2, c2)
    nc.vector.tensor_add(ro[:, :, 32:64], ta, tb)
    return ro


def build_program():
    nc = bacc.Bacc("TRN2", target_bir_lowering=False, debug=False,
                   num_devices=NCORES)
    hidT = nc.dram_tensor("hidT", [HID, T], F32, kind="ExternalInput").ap()
    ropeq = nc.dram_tensor("ropeq", [T, 128], F32, kind="ExternalInput").ap()
    ropek = nc.dram_tensor("ropek", [T, 128], F32, kind="ExternalInput").ap()
    wq = nc.dram_tensor("wq", [HID, HID], F32, kind="ExternalInput").ap()
    wkv = nc.dram_tensor("wkv", [HID, 512], F32, kind="ExternalInput").ap()
    wo = nc.dram_tensor("wo", [HID, HID], F32, kind="ExternalInput").ap()
    biasv = nc.dram_tensor("biasv", [1, 128], F32, kind="ExternalInput").ap()
    tri = nc.dram_tensor("tri", [128, 128], BF16, kind="ExternalInput").ap()
    ident = nc.dram_tensor("ident", [128, 128], F32, kind="ExternalInput").ap()
    y = nc.dram_tensor("y", [NSLOT, 128, HID], F32, kind="ExternalOutput").ap()

    with ExitStack() as ctx:
        tc = ctx.enter_context(tile.TileContext(nc))
        pers = ctx.enter_context(tc.tile_pool(name="pers", bufs=1))

        kta = pers.tile([128, NCH, 128], F32R)      # [kv0;kv1] x [t]
        ktb = pers.tile([128, NCH, 128], F32R)      # [kv2;kv3] x [t]
        v_all = pers.tile([128, NCH, HKV, 65], BF16)  # [t, kv, d|ones]
        qt_all = pers.tile([128, 8, NSLOT, 128], F32R)  # [d(2 heads), pair, slot, i]
        outt = pers.tile([64, H, NSLOT, 128], F32R)    # [d, head, slot, i]
        wkv_sb = pers.tile([128, 8, 512], F32R)
        bias_sb = pers.tile([128, NSLOT, 32], F32)
        tri_sb = pers.tile([128, 128], BF16)
        id_sb = pers.tile([128, 128], F32)
        eps_sb = pers.tile([128, 1], F32)
        nc.vector.memset(eps_sb, float(EPS))

        nc.gpsimd.dma_start(out=wkv_sb,
                            in_=wkv.rearrange("(c p) n -> p c n", p=128))
        bias_b = bass.AP(tensor=biasv.tensor, offset=0,
                         ap=[[0, 128], [32, NSLOT], [1, 32]])
        nc.sync.dma_start(out=bias_sb, in_=bias_b)
        nc.sync.dma_start(out=tri_sb, in_=tri)
        nc.sync.dma_start(out=id_sb, in_=ident)
        nc.vector.memset(v_all[:, :, :, 64:65], 1.0)

        _mark(nc, 'A1_qproj')
        # ---------------- Phase A1: q projection (own slots only) ----------
        qtd_pool = ctx.enter_context(tc.tile_pool(name="qtd", bufs=1))
        q_tds = [qtd_pool.tile([128, H, 64], F32, tag=f"q_td{sl}",
                                name=f"q_td{sl}")
                 for sl in range(NSLOT)]
        with tc.tile_pool(name="a1sb", bufs=1) as a1, \
             tc.tile_pool(name="a1wq", bufs=2) as a1w, \
             tc.tile_pool(name="a1ps", bufs=NSLOT, space="PSUM") as a1ps:
            hidq = []
            qps = []
            for sl in range(NSLOT):
                ht = a1.tile([128, 8, 128], F32R, tag=f"hidq{sl}")
                col0 = (8 * sl + 7) * 128
                src = bass.AP(tensor=hidT.tensor, offset=col0,
                              ap=[[T, 128], [128 * T, 8], [1, 128]])
                nc.gpsimd.dma_start(out=ht, in_=src)
                hidq.append(ht)
                qps.append(a1ps.tile([128, 2, 512], F32, tag="qps", name=f"qps{sl}"))
            for hc in range(8):
                wq_t = a1w.tile([128, 2, 512], F32R, tag="wq")
                nc.gpsimd.dma_start(
                    out=wq_t,
                    in_=wq[hc * 128:(hc + 1) * 128, :].rearrange(
                        "p (a n) -> p a n", a=2))
                for sl in range(NSLOT):
                    for a in range(2):
                        nc.tensor.matmul(qps[sl][:, a, :], hidq[sl][:, hc, :],
                                         wq_t[:, a, :], start=(hc == 0),
                                         stop=(hc == 7))
            for sl in range(NSLOT):
                nc.vector.tensor_copy(
                    q_tds[sl],
                    qps[sl].rearrange("p a (h d) -> p (a h) d", d=64))

        _mark(nc, 'A1b_qnorm')
        # ---------------- Phase A1b: q norm/rope/transpose -----------------
        with tc.tile_pool(name="a1bsb", bufs=2) as a1b, \
             tc.tile_pool(name="a1bps", bufs=2, space="PSUM") as a1bps:
            for sl in range(NSLOT):
                rq_t = a1b.tile([128, 4, 32], F32, tag="ropeq")
                r0 = (8 * sl + 7) * 128
                nc.sync.dma_start(
                    out=rq_t,
                    in_=ropeq[r0:r0 + 128, :].rearrange("t (a d) -> t a d", a=4))
                ro = _norm_rope(nc, a1b, q_tds[sl], rq_t, H, eps_sb)
                qtps = a1bps.tile([128, 8, 128], F32, tag="qtps")
                for p in range(8):
                    nc.tensor.transpose(
                        qtps[:, p, :],
                        ro[:, 2 * p:2 * p + 2, :].rearrange(
                            "t a d -> t (a d)"), id_sb)
                nc.vector.tensor_copy(qt_all[:, :, sl, :], qtps)

        _mark(nc, 'A2_kv')
        # ---------------- Phase A2: k/v projection + norm/rope -------------
        with tc.tile_pool(name="a2sb", bufs=2) as a2, \
             tc.tile_pool(name="a2ps", bufs=2, space="PSUM") as a2ps:
            for tch in range(NCH):
                ht = a2.tile([128, 8, 128], F32R, tag="hidt")
                src = bass.AP(tensor=hidT.tensor, offset=tch * 128,
                              ap=[[T, 128], [128 * T, 8], [1, 128]])
                nc.gpsimd.dma_start(out=ht, in_=src)
                kvps = a2ps.tile([128, 512], F32, tag="kvps")
                for hc in range(8):
                    nc.tensor.matmul(kvps, ht[:, hc, :], wkv_sb[:, hc, :],
                                     start=(hc == 0), stop=(hc == 7))
                k_td = a2.tile([128, HKV, 64], F32, tag="k_td")
                nc.vector.tensor_copy(
                    k_td, kvps[:, 0:256].rearrange("p (h d) -> p h d", d=64))
                nc.vector.tensor_copy(
                    v_all[:, tch, :, 0:64],
                    kvps[:, 256:512].rearrange("p (h d) -> p h d", d=64))
                rk_t = a2.tile([128, 4, 32], F32, tag="ropek")
                r0 = tch * 128
                nc.sync.dma_start(
                    out=rk_t,
                    in_=ropek[r0:r0 + 128, :].rearrange("t (a d) -> t a d", a=4))
                ro = _norm_rope(nc, a2, k_td, rk_t, HKV, eps_sb)
                ktps = a2ps.tile([128, 2, 128], F32, tag="ktps")
                for half in range(2):
                    nc.tensor.transpose(
                        ktps[:, half, :],
                        ro[:, 2 * half:2 * half + 2, :].rearrange(
                            "t a d -> t (a d)"), id_sb)
                nc.vector.tensor_copy(kta[:, tch, :], ktps[:, 0, :])
                nc.vector.tensor_copy(ktb[:, tch, :], ktps[:, 1, :])

        _mark(nc, 'B_attn')
        # ---------------- Phase B: attention -------------------------------
        with tc.tile_pool(name="bsb", bufs=2) as bs, \
             tc.tile_pool(name="bps", bufs=1, space="PSUM") as bps, \
             tc.tile_pool(name="bdram", bufs=2, space="DRAM") as bdram:
            for m in range(NSLOT):
                pv = bps.tile([65, H, 128], F32, tag="pv")
                niter = 8 * m + 8
                for n in range(niter):
                    et = bs.tile([128, H, 128], BF16, tag="expst")
                    for grp in range(2):
                        kt = kta if grp == 0 else ktb
                        qk = bps.tile([128, 2, 512], F32, tag=f"qk{grp}")
                        p0 = grp * 4
                        for side in range(2):
                            lo, hi = side * 64, side * 64 + 64
                            tp = (64, 0) if side else (0, 0)
                            rhs = bass.AP(
                                tensor=qt_all.tensor,
                                offset=qt_all[lo:hi, p0, m, :].offset,
                                ap=[qt_all[lo:hi, p0, m, :].ap[0],
                                    [NSLOT * 128, 4], [1, 128]])
                            nc.tensor.matmul(qk[:, side, :],
                                             kt[lo:hi, n, :], rhs,
                                             tile_position=tp,
                                             start=True, stop=True)
                        bias_arg = (bias_sb[:, m, n:n + 1]
                                    if n < niter - 1 else 0.0)
                        nc.scalar.activation(
                            et[:, grp * 8:(grp + 1) * 8, :].rearrange(
                                "p (s q) i -> p s q i", s=2),
                            qk.rearrange("p s (q i) -> p s q i", i=128),
                            AF.Exp, bias=bias_arg, scale=float(SCALE))
                    if n == niter - 1:
                        tri_b = bass.AP(tensor=tri_sb.tensor,
                                        offset=tri_sb.offset,
                                        ap=[tri_sb.ap[0], [0, H], [1, 128]])
                        nc.vector.tensor_mul(et, et, tri_b)
                    for kv in range(HKV):
                        nc.tensor.matmul(pv[:, kv * 4:(kv + 1) * 4, :],
                                         v_all[:, n, kv, :],
                                         et[:, kv * 4:(kv + 1) * 4, :],
                                         start=(n == 0), stop=(n == niter - 1))
                # normalize: outt[:, h, m, :] = pv[0:64] / pv[64]
                rec = bs.tile([1, H, 128], F32, tag="rec")
                nc.vector.reciprocal(rec, pv[64:65, :, :])
                rec_d = bdram.tile([1, H, 128], F32, tag="recd")
                nc.sync.dma_start(out=rec_d, in_=rec)
                rec_b = bass.AP(tensor=rec_d.tensor, offset=rec_d.offset,
                                ap=[[0, 64], [128, H], [1, 128]])
                rec_m = bs.tile([64, H, 128], F32, tag="recm")
                nc.gpsimd.dma_start(out=rec_m, in_=rec_b)
                nc.vector.tensor_mul(outt[:, :, m, :], pv[0:64, :, :],
                                     rec_m)

        _mark(nc, 'C_oproj')
        # ---------------- Phase C: o-projection ----------------------------
        with tc.tile_pool(name="csb", bufs=3) as cs, \
             tc.tile_pool(name="cps", bufs=1, space="PSUM") as cps:
            ops = cps.tile([128, 8, 512], F32, tag="ops")
            for h in range(H):
                wo_t = cs.tile([64, 2, 512], F32R, tag="wo")
                nc.gpsimd.dma_start(
                    out=wo_t,
                    in_=wo[h * 64:(h + 1) * 64, :].rearrange(
                        "p (a n) -> p a n", a=2))
                for m in range(NSLOT):
                    for e in range(2):
                        nc.tensor.matmul(ops[:, m * 2 + e, :],
                                         outt[:, h, m, :], wo_t[:, e, :],
                                         start=(h == 0), stop=(h == H - 1))
            for m in range(NSLOT):
                ot = cs.tile([128, 2, 512], F32, tag="oc")
                nc.vector.tensor_copy(ot, ops[:, m * 2:m * 2 + 2, :])
                nc.sync.dma_start(
                    out=y[m], in_=ot.rearrange("p a n -> p (a n)"))

    nc.compile()
    return nc


_NC_CACHE = None


def _get_program():
    global _NC_CACHE
    if _NC_CACHE is None:
        _NC_CACHE = build_program()
    return _NC_CACHE


def make_core_inputs(hidden_states, cos, sin, Wq, Wk, Wv, Wo, q_norm_w,
                     k_norm_w):
    """Host-side marshalling: per-core permutation + layout prep."""
    hid = np.asarray(hidden_states, dtype=np.float32)[0]      # [T, HID]
    cosg = np.asarray(cos, dtype=np.float32)[0]               # [T, 32]
    sing = np.asarray(sin, dtype=np.float32)[0]
    qw = np.asarray(q_norm_w, dtype=np.float32)
    kw = np.asarray(k_norm_w, dtype=np.float32)

    def rope_tab(w):
        return np.concatenate([cosg * w[None, :32], sing * w[None, 32:],
                               sing * w[None, :32], cosg * w[None, 32:]],
                              axis=1)                          # [T, 128]

    ropeq_g = rope_tab(qw)
    ropek_g = rope_tab(kw)
    wq_r = np.ascontiguousarray(
        np.asarray(Wq, dtype=np.float32).reshape(HID, H, D)[:, HORD, :]
        .reshape(HID, HID))
    wkv_g = np.ascontiguousarray(
        np.concatenate([np.asarray(Wk, dtype=np.float32),
                        np.asarray(Wv, dtype=np.float32)], axis=1))
    wo_g = np.ascontiguousarray(np.asarray(Wo, dtype=np.float32))
    tri_g = np.triu(np.ones((128, 128))).astype(ml_dtypes.bfloat16)
    id_g = np.eye(128, dtype=np.float32)

    in_maps = []
    perms = []
    for c in range(NCORES):
        perm = np.arange(NCH)
        for m in range(NSLOT):
            a, b = 8 * m + c, 8 * m + 7
            perm[[a, b]] = perm[[b, a]]
        perms.append(perm)
        hp = hid.reshape(NCH, 128, HID)[perm].reshape(T, HID)
        hidT_c = np.ascontiguousarray(hp.T)
        rq_c = np.ascontiguousarray(
            ropeq_g.reshape(NCH, 128, 128)[perm].reshape(T, 128))
        rk_c = np.ascontiguousarray(
            ropek_g.reshape(NCH, 128, 128)[perm].reshape(T, 128))
        bias_c = np.zeros((NSLOT, 32), dtype=np.float32)
        for m in range(NSLOT):
            for n in range(8 * m + 7):
                if n >= 8 * m + c:
                    bias_c[m, n] = NEG
        in_maps.append(dict(
            hidT=hidT_c, ropeq=rq_c, ropek=rk_c, wq=wq_r, wkv=wkv_g,
            wo=wo_g, biasv=bias_c.reshape(1, 128), tri=tri_g, ident=id_g))
    return in_maps, perms


def kernel(hidden_states, cos, sin, Wq, Wk, Wv, Wo, q_norm_w, k_norm_w):
    nc = _get_program()
    in_maps, _ = make_core_inputs(hidden_states, cos, sin, Wq, Wk, Wv, Wo,
                                  q_norm_w, k_norm_w)
    res = run_bass_kernel_spmd(nc, in_maps, core_ids=list(range(NCORES)))
    out = np.empty((T, HID), dtype=np.float32)
    for c in range(NCORES):
        yc = res.results[c]["y"]                   # [NSLOT, 128, HID]
        for m in range(NSLOT):
            blk = 8 * m + c
            out[blk * 128:(blk + 1) * 128] = yc[m]
    return out.reshape(B, T, HID)


# revision 23
# speedup vs baseline: 117.6756x; 117.6756x over previous
"""Trainium2 Bass kernel for nn_CATDecoderAttention (GQA causal attention block).

Computation (fp32 reference): q/k/v projections -> per-head RMSNorm on q,k ->
RoPE on q,k -> causal GQA attention (16 q heads, 4 kv heads, D=64) -> o-proj.

Sharding: sequence-parallel over queries. Each of the 8 cores owns 4 query
chunks of 128 rows (chunk set {8m+c : m=0..3} for core c), chosen so the causal
work is identical across cores (SPMD single program). A per-core permutation of
the 4096 t-rows (swapping blocks 8m+c <-> 8m+7 within each group of 8) puts the
core's own (diagonal) chunk at the fixed position 8m+7, so the diagonal
tri-mask sits at a compile-time program position; remaining causal differences
between cores are handled by a per-core bias table feeding exp() (whole-block
masking, free via the activation bias operand). K/V work is replicated across
cores; outputs are disjoint row-slices, so no collectives are needed, and the
host scatters rows back.

Layouts: projections consume host-pretransposed hidden^T and produce q/k in
[t, d] (norm/RoPE domain), then PE-transpose to [d, t] for QK^T. Scores are
computed transposed (S^T[j, i]) so PV needs no transpose; a constant ones
column appended to V yields the softmax denominators for free. fp32r matmuls
(1 cycle/row, ~1e-4 matmul error) everywhere except the PV stage (bf16).
"""

import sys
import numpy as np

sys.path.insert(0, "/opt/trn_rl_repo")

import ml_dtypes  # noqa: E402
from contextlib import ExitStack  # noqa: E402

import concourse.bass as bass  # noqa: E402
import concourse.tile as tile  # noqa: E402
from concourse import bacc, mybir  # noqa: E402
from concourse.bass_utils import run_bass_kernel_spmd  # noqa: E402

F32 = mybir.dt.float32
F32R = mybir.dt.float32r
BF16 = mybir.dt.bfloat16
AX = mybir.AxisListType.X
AF = mybir.ActivationFunctionType
ALU = mybir.AluOpType

B, T, HID = 1, 4096, 1024
H, HKV, D = 16, 4, 64
G = H // HKV
EPS = 1e-6
SCALE = 1.0 / np.sqrt(D)
NCORES = 8
NSLOT = 4           # own query chunks per core
NCH = T // 128      # 32 t-blocks
NEG = -30000.0      # exp(x + NEG) == 0 for any reachable score

# q head order in the q[t,d] working layout: pair p = heads (HORD[2p], HORD[2p+1])
# chosen so pair sides have kv groups (0,1) resp. (2,3) for QK row-tiling.
HORD = [0, 4, 1, 5, 2, 6, 3, 7, 8, 12, 9, 13, 10, 14, 11, 15]

PHASE_MARKS = []  # (phase_name, first_instruction_id) — filled by build_program


def _mark(nc, name):
    PHASE_MARKS.append((name, nc._state.next_id()))


def _bcast_free(tile_ap, idx, count, width):
    """AP reading tile_ap[:, idx, 0:width] broadcast `count` times along a
    middle free dim of step 0."""
    base = tile_ap[:, idx, 0:width]
    return bass.AP(tensor=base.tensor, offset=base.offset,
                   ap=[base.ap[0], [0, count], [1, width]])


def _rstd_batch(nc, pool, ss, nh, eps_sb, tag):
    """rstd = exp(-0.5*ln(mean+eps)) over a whole batch of sum-of-squares in
    one Ln + one Exp — Ln and Exp share an ACT table set with phase B's exp,
    and batching avoids per-block table thrash/op overhead."""
    st = pool.tile(list(ss.shape), F32, tag=f"{tag}_st", name=f"{tag}_st")
    nc.scalar.activation(st, ss, AF.Ln, bias=eps_sb, scale=1.0 / D)
    rs = pool.tile(list(ss.shape), F32, tag=f"{tag}_rs", name=f"{tag}_rs")
    nc.scalar.activation(rs, st, AF.Exp, scale=-0.5)
    return rs


def _norm_rope(nc, pool, x_td, tab, nh, rs_ap, out_dt=F32):
    """Apply RMSNorm scale (rstd given as [128, nh] AP) + RoPE.
    x_td: [128, nh, 64] f32 SBUF. tab: [128, 4, 32] f32 (C1|S1|S2|C2)."""
    nm = pool.tile([128, nh, 64], F32, tag="nr_nm")
    rs_b = bass.AP(tensor=rs_ap.tensor, offset=rs_ap.offset,
                   ap=[rs_ap.ap[0], list(rs_ap.ap[-1]), [0, 64]])
    nc.vector.tensor_mul(nm, x_td, rs_b)
    ro = pool.tile([128, nh, 64], out_dt, tag="nr_ro")
    ta = pool.tile([128, nh, 32], F32, tag="nr_ta")
    tb = pool.tile([128, nh, 32], F32, tag="nr_tb")
    x1 = nm[:, :, 0:32]
    x2 = nm[:, :, 32:64]
    c1 = _bcast_free(tab, 0, nh, 32)
    s1 = _bcast_free(tab, 1, nh, 32)
    s2 = _bcast_free(tab, 2, nh, 32)
    c2 = _bcast_free(tab, 3, nh, 32)
    nc.vector.tensor_mul(ta, x1, c1)
    nc.vector.tensor_mul(tb, x2, s1)
    nc.vector.tensor_sub(ro[:, :, 0:32], ta, tb)
    nc.vector.tensor_mul(ta, x1, s2)
    nc.vector.tensor_mul(tb, x2, c2)
    nc.vector.tensor_add(ro[:, :, 32:64], ta, tb)
    return ro


def build_program(reps=1):
    nc = bacc.Bacc("TRN2", target_bir_lowering=False, debug=False,
                   num_devices=NCORES)
    hidT = nc.dram_tensor("hidT", [HID, T], F32, kind="ExternalInput").ap()
    ropeq = nc.dram_tensor("ropeq", [T, 128], F32, kind="ExternalInput").ap()
    ropek = nc.dram_tensor("ropek", [T, 128], F32, kind="ExternalInput").ap()
    wq = nc.dram_tensor("wq", [HID, HID], F32, kind="ExternalInput").ap()
    wkv = nc.dram_tensor("wkv", [HID, 512], F32, kind="ExternalInput").ap()
    wo = nc.dram_tensor("wo", [HID, HID], F32, kind="ExternalInput").ap()
    biasv = nc.dram_tensor("biasv", [1, 128], F32, kind="ExternalInput").ap()
    tri = nc.dram_tensor("tri", [128, 128], BF16, kind="ExternalInput").ap()
    ident = nc.dram_tensor("ident", [128, 128], F32, kind="ExternalInput").ap()
    y = nc.dram_tensor("y", [NSLOT, 128, HID], F32, kind="ExternalOutput").ap()

    with ExitStack() as octx:
      tc = octx.enter_context(tile.TileContext(nc))
      for _rep in range(reps):
       with ExitStack() as ctx:
        pers = ctx.enter_context(tc.tile_pool(name="pers", bufs=1))

        # k/v/out tensors are segmented per slot (4 tiles of 8 t-blocks each)
        # so phase-B(m) reads never false-WAR against segment m+1's writes
        # (Tile dependency tracking is effectively tile-granular).
        kta_s = [pers.tile([128, 8, 128], F32R, name=f"kta{s}")
                 for s in range(NSLOT)]
        ktb_s = [pers.tile([128, 8, 128], F32R, name=f"ktb{s}")
                 for s in range(NSLOT)]
        v_s = [pers.tile([128, 8, HKV, 65], BF16, name=f"v{s}")
               for s in range(NSLOT)]
        outt_s = [pers.tile([64, H, 128], F32R, name=f"outt{s}")
                  for s in range(NSLOT)]
        qt_all = pers.tile([128, 8, NSLOT, 128], F32R)  # [d(2 heads), pair, slot, i]
        wkv_sb = pers.tile([128, 8, 512], F32R)
        bias_sb = pers.tile([128, NSLOT, 32], F32)
        tri_sb = pers.tile([128, 128], BF16)
        id_sb = pers.tile([128, 128], F32)
        eps_sb = pers.tile([128, 1], F32)
        nc.vector.memset(eps_sb, float(EPS))

        nc.gpsimd.dma_start(out=wkv_sb,
                            in_=wkv.rearrange("(c p) n -> p c n", p=128))
        bias_b = bass.AP(tensor=biasv.tensor, offset=0,
                         ap=[[0, 128], [32, NSLOT], [1, 32]])
        nc.sync.dma_start(out=bias_sb, in_=bias_b)
        nc.sync.dma_start(out=tri_sb, in_=tri)
        nc.sync.dma_start(out=id_sb, in_=ident)
        for s in range(NSLOT):
            nc.vector.memset(v_s[s][:, :, :, 64:65], 1.0)

        _mark(nc, 'A1_qproj')
        # ---------------- Phase A1: q projection (own slots only) ----------
        qtd_pool = ctx.enter_context(tc.tile_pool(name="qtd", bufs=1))
        q_tds = [qtd_pool.tile([128, H, 64], F32, tag=f"q_td{sl}",
                                name=f"q_td{sl}")
                 for sl in range(NSLOT)]
        with tc.tile_pool(name="a1sb", bufs=1) as a1, \
             tc.tile_pool(name="a1wq", bufs=2) as a1w, \
             tc.tile_pool(name="a1ps", bufs=NSLOT, space="PSUM") as a1ps:
            hidq = []
            qps = []
            for sl in range(NSLOT):
                ht = a1.tile([128, 8, 128], F32R, tag=f"hidq{sl}")
                col0 = (8 * sl + 7) * 128
                src = bass.AP(tensor=hidT.tensor, offset=col0,
                              ap=[[T, 128], [128 * T, 8], [1, 128]])
                nc.gpsimd.dma_start(out=ht, in_=src)
                hidq.append(ht)
                qps.append(a1ps.tile([128, 2, 512], F32, tag="qps", name=f"qps{sl}"))
            for hc in range(8):
                wq_t = a1w.tile([128, 2, 512], F32R, tag="wq")
                nc.gpsimd.dma_start(
                    out=wq_t,
                    in_=wq[hc * 128:(hc + 1) * 128, :].rearrange(
                        "p (a n) -> p a n", a=2))
                for sl in range(NSLOT):
                    for a in range(2):
                        nc.tensor.matmul(qps[sl][:, a, :], hidq[sl][:, hc, :],
                                         wq_t[:, a, :], start=(hc == 0),
                                         stop=(hc == 7))
            for sl in range(NSLOT):
                nc.vector.tensor_copy(
                    q_tds[sl],
                    qps[sl].rearrange("p a (h d) -> p (a h) d", d=64))

        # ------- Phases A1b/A2/B interleaved. Program order:
        # ------- seg(0), A1b, B(0), seg(1), B(1), ... so the scheduler can
        # ------- hide segment PE/DVE work under B's ACT-bound stretches.
        # ------- PSUM budget: a2ps 2 + qk 4 + pv 2 = 8 banks (A1b's qtps
        # ------- pool closes before the B pools open).
        ictx = ExitStack()
        a2 = ictx.enter_context(tc.tile_pool(name="a2sb", bufs=2))
        a2ps = ictx.enter_context(tc.tile_pool(name="a2ps", bufs=2,
                                               space="PSUM"))
        bdram = ictx.enter_context(tc.tile_pool(name="bdram", bufs=2,
                                                space="DRAM"))

        def emit_seg(m):
            # ---- A2 segment: k/v projection+norm for t-blocks 8m..8m+7 ----
            k_td = a2.tile([128, 8, HKV, 64], F32, tag="k_td",
                           name=f"k_td{m}")
            ssk = a2.tile([128, 8, HKV], F32, tag="ssk", name=f"ssk{m}")
            for tci in range(8):
                tch = 8 * m + tci
                ht = a2.tile([128, 8, 128], F32R, tag="hidt",
                             name=f"hidt{tch}")
                src = bass.AP(tensor=hidT.tensor, offset=tch * 128,
                              ap=[[T, 128], [128 * T, 8], [1, 128]])
                nc.gpsimd.dma_start(out=ht, in_=src)
                kvps = a2ps.tile([128, 512], F32, tag="a2ps",
                                 name=f"kvps{tch}")
                for hc in range(8):
                    nc.tensor.matmul(kvps, ht[:, hc, :], wkv_sb[:, hc, :],
                                     start=(hc == 0), stop=(hc == 7))
                nc.vector.tensor_copy(
                    k_td[:, tci, :, :],
                    kvps[:, 0:256].rearrange("p (h d) -> p h d", d=64))
                nc.vector.tensor_copy(
                    v_s[m][:, tci, :, 0:64],
                    kvps[:, 256:512].rearrange("p (h d) -> p h d", d=64))
                sqk = a2.tile([128, HKV, 64], F32, tag="sqk",
                              name=f"sqk{tch}")
                nc.vector.tensor_mul(sqk, k_td[:, tci, :, :],
                                     k_td[:, tci, :, :])
                nc.vector.reduce_sum(ssk[:, tci, :], sqk, axis=AX)
            rsk = _rstd_batch(nc, a2, ssk, HKV, eps_sb, f"k{m}")
            for tci in range(8):
                tch = 8 * m + tci
                rk_t = a2.tile([128, 4, 32], F32, tag="ropek",
                               name=f"rk_t{tch}")
                r0 = tch * 128
                nc.sync.dma_start(
                    out=rk_t,
                    in_=ropek[r0:r0 + 128, :].rearrange(
                        "t (a d) -> t a d", a=4))
                ro = _norm_rope(nc, a2, k_td[:, tci, :, :], rk_t, HKV,
                                rsk[:, tci, :])
                ktps = a2ps.tile([128, 2, 128], F32, tag="a2ps",
                                 name=f"ktps{tch}")
                for half in range(2):
                    nc.tensor.transpose(
                        ktps[:, half, :],
                        ro[:, 2 * half:2 * half + 2, :].rearrange(
                            "t a d -> t (a d)"), id_sb)
                nc.vector.tensor_copy(kta_s[m][:, tci, :], ktps[:, 0, :])
                nc.vector.tensor_copy(ktb_s[m][:, tci, :], ktps[:, 1, :])

        _mark(nc, 'A2_seg0')
        emit_seg(0)

        _mark(nc, 'A1b_qnorm')
        # ---------------- Phase A1b: q norm/rope/transpose -----------------
        with tc.tile_pool(name="a1bsb", bufs=2) as a1b, \
             tc.tile_pool(name="a1bps", bufs=2, space="PSUM") as a1bps:
            ssq = a1b.tile([128, NSLOT, H], F32, tag="ssq", bufs=1)
            for sl in range(NSLOT):
                sq = a1b.tile([128, H, 64], F32, tag="nr_sq")
                nc.vector.tensor_mul(sq, q_tds[sl], q_tds[sl])
                nc.vector.reduce_sum(ssq[:, sl, :], sq, axis=AX)
            rsq = _rstd_batch(nc, a1b, ssq, H, eps_sb, "q")
            for sl in range(NSLOT):
                rq_t = a1b.tile([128, 4, 32], F32, tag="ropeq")
                r0 = (8 * sl + 7) * 128
                nc.sync.dma_start(
                    out=rq_t,
                    in_=ropeq[r0:r0 + 128, :].rearrange("t (a d) -> t a d", a=4))
                ro = _norm_rope(nc, a1b, q_tds[sl], rq_t, H, rsq[:, sl, :])
                qtps = a1bps.tile([128, 8, 128], F32, tag="qtps")
                for p in range(8):
                    nc.tensor.transpose(
                        qtps[:, p, :],
                        ro[:, 2 * p:2 * p + 2, :].rearrange(
                            "t a d -> t (a d)"), id_sb)
                nc.vector.tensor_copy(qt_all[:, :, sl, :], qtps)

        _mark(nc, 'AB_inter')
        with tc.tile_pool(name="bsb", bufs=3) as bs, \
             tc.tile_pool(name="bps", bufs=1, space="PSUM") as bps:
            tri_b = bass.AP(tensor=tri_sb.tensor, offset=tri_sb.offset,
                            ap=[tri_sb.ap[0], [0, 8], [1, 128]])
            for m in range(NSLOT):
                if m > 0:
                    emit_seg(m)
                # ---- B(m): attention for own slot m, in 2 head-waves ----
                niter = 8 * m + 8
                for w in range(2):
                    kt_s = kta_s if w == 0 else ktb_s
                    p0 = w * 4
                    pv = bps.tile([65, 8, 128], F32, tag="pv")
                    for n in range(niter):
                        seg, col = n // 8, n % 8
                        et = bs.tile([128, 8, 128], BF16, tag="expst")
                        qk = bps.tile([128, 2, 512], F32, tag="qk", bufs=2)
                        for side in range(2):
                            lo, hi = side * 64, side * 64 + 64
                            tp = (64, 0) if side else (0, 0)
                            rhs = bass.AP(
                                tensor=qt_all.tensor,
                                offset=qt_all[lo:hi, p0, m, :].offset,
                                ap=[qt_all[lo:hi, p0, m, :].ap[0],
                                    [NSLOT * 128, 4], [1, 128]])
                            nc.tensor.matmul(qk[:, side, :],
                                             kt_s[seg][lo:hi, col, :], rhs,
                                             tile_position=tp,
                                             start=True, stop=True)
                        bias_arg = (bias_sb[:, m, n:n + 1]
                                    if n < niter - 1 else 0.0)
                        nc.scalar.activation(
                            et.rearrange("p (s q) i -> p s q i", s=2),
                            qk.rearrange("p s (q i) -> p s q i", i=128),
                            AF.Exp, bias=bias_arg, scale=float(SCALE))
                        if n == niter - 1:
                            nc.vector.tensor_mul(et, et, tri_b)
                        for kvi in range(2):
                            kv = 2 * w + kvi
                            nc.tensor.matmul(
                                pv[:, kvi * 4:(kvi + 1) * 4, :],
                                v_s[seg][:, col, kv, :],
                                et[:, kvi * 4:(kvi + 1) * 4, :],
                                start=(n == 0), stop=(n == niter - 1))
                    # Drain pv to SBUF immediately (frees the PSUM banks for
                    # the next wave), then normalize off the critical path.
                    pvr = bs.tile([65, 8, 128], F32, tag="pvraw")
                    nc.vector.tensor_copy(pvr, pv)
                    rec = bs.tile([1, 8, 128], F32, tag="rec")
                    nc.vector.reciprocal(rec, pvr[64:65, :, :])
                    rec_d = bdram.tile([1, 8, 128], F32, tag="recd")
                    nc.sync.dma_start(out=rec_d, in_=rec)
                    rec_b = bass.AP(tensor=rec_d.tensor, offset=rec_d.offset,
                                    ap=[[0, 64], [128, 8], [1, 128]])
                    rec_m = bs.tile([64, 8, 128], F32, tag="recm")
                    nc.gpsimd.dma_start(out=rec_m, in_=rec_b)
                    nc.vector.tensor_mul(outt_s[m][:, 8 * w:8 * w + 8, :],
                                         pvr[0:64, :, :], rec_m)

        ictx.close()

        _mark(nc, 'C_oproj')
        # ---------------- Phase C: o-projection ----------------------------
        with tc.tile_pool(name="csb", bufs=3) as cs, \
             tc.tile_pool(name="cps", bufs=1, space="PSUM") as cps:
            ops = cps.tile([128, 8, 512], F32, tag="ops")
            for h in range(H):
                wo_t = cs.tile([64, 2, 512], F32R, tag="wo")
                nc.gpsimd.dma_start(
                    out=wo_t,
                    in_=wo[h * 64:(h + 1) * 64, :].rearrange(
                        "p (a n) -> p a n", a=2))
                for m in range(NSLOT):
                    for e in range(2):
                        nc.tensor.matmul(ops[:, m * 2 + e, :],
                                         outt_s[m][:, h, :], wo_t[:, e, :],
                                         start=(h == 0), stop=(h == H - 1))
            for m in range(NSLOT):
                ot = cs.tile([128, 2, 512], F32, tag="oc")
                nc.vector.tensor_copy(ot, ops[:, m * 2:m * 2 + 2, :])
                nc.sync.dma_start(
                    out=y[m], in_=ot.rearrange("p a n -> p (a n)"))

    nc.compile()
    return nc


_NC_CACHE = None


def _get_program():
    global _NC_CACHE
    if _NC_CACHE is None:
        _NC_CACHE = build_program()
    return _NC_CACHE


def make_core_inputs(hidden_states, cos, sin, Wq, Wk, Wv, Wo, q_norm_w,
                     k_norm_w):
    """Host-side marshalling: per-core permutation + layout prep."""
    hid = np.asarray(hidden_states, dtype=np.float32)[0]      # [T, HID]
    cosg = np.asarray(cos, dtype=np.float32)[0]               # [T, 32]
    sing = np.asarray(sin, dtype=np.float32)[0]
    qw = np.asarray(q_norm_w, dtype=np.float32)
    kw = np.asarray(k_norm_w, dtype=np.float32)

    def rope_tab(w):
        return np.concatenate([cosg * w[None, :32], sing * w[None, 32:],
                               sing * w[None, :32], cosg * w[None, 32:]],
                              axis=1)                          # [T, 128]

    ropeq_g = rope_tab(qw)
    ropek_g = rope_tab(kw)
    wq_r = np.ascontiguousarray(
        np.asarray(Wq, dtype=np.float32).reshape(HID, H, D)[:, HORD, :]
        .reshape(HID, HID))
    wkv_g = np.ascontiguousarray(
        np.concatenate([np.asarray(Wk, dtype=np.float32),
                        np.asarray(Wv, dtype=np.float32)], axis=1))
    wo_g = np.ascontiguousarray(np.asarray(Wo, dtype=np.float32))
    tri_g = np.triu(np.ones((128, 128))).astype(ml_dtypes.bfloat16)
    id_g = np.eye(128, dtype=np.float32)

    in_maps = []
    perms = []
    for c in range(NCORES):
        perm = np.arange(NCH)
        for m in range(NSLOT):
            a, b = 8 * m + c, 8 * m + 7
            perm[[a, b]] = perm[[b, a]]
        perms.append(perm)
        hp = hid.reshape(NCH, 128, HID)[perm].reshape(T, HID)
        hidT_c = np.ascontiguousarray(hp.T)
        rq_c = np.ascontiguousarray(
            ropeq_g.reshape(NCH, 128, 128)[perm].reshape(T, 128))
        rk_c = np.ascontiguousarray(
            ropek_g.reshape(NCH, 128, 128)[perm].reshape(T, 128))
        bias_c = np.zeros((NSLOT, 32), dtype=np.float32)
        for m in range(NSLOT):
            for n in range(8 * m + 7):
                if n >= 8 * m + c:
                    bias_c[m, n] = NEG
        in_maps.append(dict(
            hidT=hidT_c, ropeq=rq_c, ropek=rk_c, wq=wq_r, wkv=wkv_g,
            wo=wo_g, biasv=bias_c.reshape(1, 128), tri=tri_g, ident=id_g))
    return in_maps, perms


def kernel(hidden_states, cos, sin, Wq, Wk, Wv, Wo, q_norm_w, k_norm_w):
    nc = _get_program()
    in_maps, _ = make_core_inputs(hidden_states, cos, sin, Wq, Wk, Wv, Wo,
                                  q_norm_w, k_norm_w)
    res = run_bass_kernel_spmd(nc, in_maps, core_ids=list(range(NCORES)))
    out = np.empty((T, HID), dtype=np.float32)
    for c in range(NCORES):
        yc = res.results[c]["y"]                   # [NSLOT, 128, HID]
        for m in range(NSLOT):
            blk = 8 * m + c
            out[blk * 128:(blk + 1) * 128] = yc[m]
    return out.reshape(B, T, HID)
